# revision 4
# baseline (speedup 1.0000x reference)
"""DINOv2 LoRA featurizer histogram-binning kernel for TRN2 (8 NeuronCores).

Reference computation (per sample):
  x: [37, 37, 384] -> bx = x^T [384, 37, 37]
  pool0 = bx, pool1 = AvgPool2d(3, stride 1, pad 1, count_include_pad=False)
  17 bins = border-clamped shifts of pool0 (9 bins, offsets +-1) and
  pool1 (8 bins, offsets +-3); bins 17..28 of 29 are zero.
  out = [29*384, 37, 37] with channel c = bin*384 + feature.

Sharding: pure data parallel, sample b -> core b (B == 8 == n_cores).

Device strategy (per core):
  - channels on partitions (3 tiles of 128), spatial flattened in the free dim
  - pool1 via separable 3x3 sum using FLAT shifted adds + tiny edge-column
    fixes (strided ops are ~3x slower than flat on DVE), then * 1/count
  - for each (pool k, dx) build a column-pre-shifted, row-replicated-padded
    array R[k][dxi]; every bin is then a CONTIGUOUS row-window of some R,
    so bin stores are pure DMAs with no per-bin compute
  - bins grouped by (k, dy): one DMA moves 2-3 bins x 3 channel tiles
  - the 12 zero bins are never written: ExternalOutput buffers are
    pre-zeroed by the runner on both the native and PJRT paths.
"""

import numpy as np

B = 8
W = 37          # spatial side
WW = W * W      # 1369
D = 384
P = 128
ST = D // P     # 3 channel tiles of 128
NBINS = 29
PAD0, PAD1 = 1, 3
R0ROWS = W + 2 * PAD0   # 39
R1ROWS = W + 2 * PAD1   # 43

_CACHE = {}


def _inv_count() -> np.ndarray:
    """1 / (3x3 valid-neighbor count) per spatial position, tiled to [P, WW]."""
    cnt = np.zeros((W, W), np.float64)
    for dy in (-1, 0, 1):
        for dx in (-1, 0, 1):
            cnt[max(0, dy) : W + min(0, dy), max(0, dx) : W + min(0, dx)] += 1.0
    inv = (1.0 / cnt).astype(np.float32).reshape(WW)
    return np.broadcast_to(inv, (P, WW)).copy()


def _build_nc():
    import concourse.bass as bass  # noqa: F401
    import concourse.tile as tile
    from concourse import bacc, mybir
    from contextlib import ExitStack

    f32 = mybir.dt.float32
    nc = bacc.Bacc("TRN2", target_bir_lowering=False, debug=False)

    xt = nc.declare_dram_parameter("xt", [ST, P, WW], f32, isOutput=False)
    invcnt = nc.declare_dram_parameter("invcnt", [P, WW], f32, isOutput=False)
    out = nc.declare_dram_parameter("out", [NBINS, ST, P, WW], f32, isOutput=True)

    with tile.TileContext(nc) as tc, ExitStack() as ctx:
        perm = ctx.enter_context(tc.tile_pool(name="perm", bufs=1))
        tmp = ctx.enter_context(tc.tile_pool(name="tmp", bufs=2))

        inv = perm.tile([P, WW], f32, name="inv")
        nc.sync.dma_start(inv[:, :], invcnt.ap())

        # R0: [dxi, t, 39, 37]  (pad 1, col-shift dx in {-1,0,+1})
        # R1: [dxi, t, 43, 37]  (pad 3, col-shift dx in {-3,0,+3})
        R0 = perm.tile([P, 3, ST, R0ROWS, W], f32, name="R0")
        R1 = perm.tile([P, 3, ST, R1ROWS, W], f32, name="R1")

        # x -> center rows of R0[dxi=1] for all 3 channel tiles, one DMA
        nc.sync.dma_start(
            R0[:, 1, :, PAD0 : PAD0 + W, :].rearrange("p t a b -> p t (a b)"),
            xt.ap().transpose([1, 0, 2]),
        )

        def flatc(ap5, dxi, t, pad):
            # center of one R plane as flat [P, WW] (rows are full-width ->
            # the row window is contiguous per partition)
            return ap5[:, dxi, t, pad : pad + W, :].rearrange("p a b -> p (a b)")

        def c3(ap5, dxi, t, pad):
            # center of one R plane as [P, W, W]
            return ap5[:, dxi, t, pad : pad + W, :]

        for t in range(ST):
            X = flatc(R0, 1, t, PAD0)        # [P, WW] flat view of input
            X3 = c3(R0, 1, t, PAD0)          # [P, W, W]

            # --- column pass: T[i,j] = sum_dx X[i, j+dx], zero outside ---
            T = tmp.tile([P, WW], f32, name="T", tag="T")
            T3 = T.rearrange("p (a b) -> p a b", a=W, b=W)
            nc.vector.tensor_add(T[:, 0 : WW - 1], X[:, 0 : WW - 1], X[:, 1:WW])
            nc.vector.tensor_copy(T[:, WW - 1 : WW], X[:, WW - 1 : WW])
            nc.vector.tensor_add(T[:, 1:WW], T[:, 1:WW], X[:, 0 : WW - 1])
            # fix row-boundary wrap at columns 0 and W-1
            nc.vector.tensor_add(T3[:, :, 0], X3[:, :, 0], X3[:, :, 1])
            nc.vector.tensor_add(T3[:, :, W - 1], X3[:, :, W - 2], X3[:, :, W - 1])

            # --- row pass: S[i,j] = sum_dy T[i+dy,j] (flat is exact) ---
            S = tmp.tile([P, WW], f32, name="S", tag="S")
            nW = WW - W
            nc.vector.tensor_add(S[:, 0:nW], T[:, 0:nW], T[:, W:WW])
            nc.vector.tensor_copy(S[:, nW:WW], T[:, nW:WW])
            nc.vector.tensor_add(S[:, W:WW], S[:, W:WW], T[:, 0:nW])

            # --- pool1 = S * invcnt -> center of R1[dxi=1] ---
            nc.vector.tensor_mul(flatc(R1, 1, t, PAD1), S[:, :], inv[:, :])

            # --- column-shifted copies (flat shift + edge-column fixes) ---
            P1 = flatc(R1, 1, t, PAD1)
            P13 = c3(R1, 1, t, PAD1)
            # R0 dx=-1: dest[:,j] = X[:, max(j-1,0)] -> flat dest[1:] = X[:-1]
            d = flatc(R0, 0, t, PAD0)
            d3 = c3(R0, 0, t, PAD0)
            nc.scalar.copy(d[:, 1:WW], X[:, 0 : WW - 1])
            nc.scalar.copy(d3[:, :, 0], X3[:, :, 0])
            # R0 dx=+1: dest[:,j] = X[:, min(j+1,W-1)] -> flat dest[:-1] = X[1:]
            d = flatc(R0, 2, t, PAD0)
            d3 = c3(R0, 2, t, PAD0)
            nc.gpsimd.tensor_copy(d[:, 0 : WW - 1], X[:, 1:WW])
            nc.gpsimd.tensor_copy(d3[:, :, W - 1], X3[:, :, W - 1])
            # R1 dx=-3
            d = flatc(R1, 0, t, PAD1)
            d3 = c3(R1, 0, t, PAD1)
            nc.scalar.copy(d[:, 3:WW], P1[:, 0 : WW - 3])
            nc.scalar.copy(
                d3[:, :, 0:3],
                P13[:, :, 0].unsqueeze(2).broadcast_to([P, W, 3]),
            )
            # R1 dx=+3
            d = flatc(R1, 2, t, PAD1)
            d3 = c3(R1, 2, t, PAD1)
            nc.gpsimd.tensor_copy(d[:, 0 : WW - 3], P1[:, 3:WW])
            nc.gpsimd.tensor_copy(
                d3[:, :, W - 3 : W],
                P13[:, :, W - 1].unsqueeze(2).broadcast_to([P, W, 3]),
            )

            # --- row-replica padding for all planes ---
            def cp(eng, dst, src):
                if eng is nc.scalar:
                    eng.copy(dst, src)
                else:
                    eng.tensor_copy(dst, src)

            for dxi in range(3):
                eng = [nc.scalar, nc.vector, nc.gpsimd][dxi]
                # R0: one row top/bottom
                r = R0[:, dxi, t]
                cp(eng, r[:, 0, :], r[:, 1, :])
                cp(eng, r[:, R0ROWS - 1, :], r[:, R0ROWS - 2, :])
                # R1: three rows top/bottom (broadcast one row)
                r = R1[:, dxi, t]
                cp(eng, r[:, 0:PAD1, :], r[:, PAD1, :].unsqueeze(1).broadcast_to([P, PAD1, W]))
                cp(
                    eng,
                    r[:, PAD1 + W : R1ROWS, :],
                    r[:, PAD1 + W - 1, :].unsqueeze(1).broadcast_to([P, PAD1, W]),
                )

        # --- bin stores: one DMA per (k, dy) group ---
        # k=0 bins p = 3*(dy+1)+(dx+1), dx-major inner: all 3 dxi, 3 t
        for r_i, dy in enumerate((-1, 0, 1)):
            src = R0[:, :, :, PAD0 + dy : PAD0 + dy + W, :].rearrange(
                "p x t a b -> p (x t) (a b)"
            )
            dst = out.ap()[3 * r_i : 3 * r_i + 3].transpose([2, 0, 1, 3]).rearrange(
                "p x t e -> p (x t) e"
            )
            nc.sync.dma_start(dst, src)
        # k=1 bins 9..16: dy in {-3,0,3}, dx in {-3,0,3} minus (0,0)
        k1_groups = [(-3, (0, 1, 2), 9), (0, (0, 2), 12), (3, (0, 1, 2), 14)]
        for dy, dxis, p0 in k1_groups:
            for j, dxi in enumerate(dxis):
                src = R1[:, dxi, :, PAD1 + dy : PAD1 + dy + W, :].rearrange(
                    "p t a b -> p t (a b)"
                )
                dst = out.ap()[p0 + j].transpose([1, 0, 2])
                nc.sync.dma_start(dst, src)

    nc.compile()
    return nc


def get_nc():
    if "nc" not in _CACHE:
        _CACHE["nc"] = _build_nc()
    return _CACHE["nc"]


def make_in_maps(x: np.ndarray):
    x = np.ascontiguousarray(x, dtype=np.float32)
    assert x.shape == (B, W, W, D), x.shape
    inv = _inv_count()
    maps = []
    for b in range(B):
        xt = x[b].transpose(2, 0, 1).reshape(ST, P, WW)
        maps.append({"xt": np.ascontiguousarray(xt), "invcnt": inv})
    return maps


def run(x: np.ndarray, **kw):
    from concourse.bass_utils import run_bass_kernel_spmd

    nc = get_nc()
    res = run_bass_kernel_spmd(nc, make_in_maps(x), core_ids=list(range(B)), **kw)
    outs = np.stack([res.results[b]["out"].reshape(NBINS * D, W, W) for b in range(B)])
    return outs, res


def kernel(x: np.ndarray) -> np.ndarray:
    outs, _ = run(x)
    return outs


# revision 8
# speedup vs baseline: 1.3980x; 1.3980x over previous
"""DINOv2 LoRA featurizer histogram-binning kernel for TRN2 (8 NeuronCores).

Reference computation (per sample):
  x: [37, 37, 384] -> bx = x^T [384, 37, 37]
  pool0 = bx, pool1 = AvgPool2d(3, stride 1, pad 1, count_include_pad=False)
  17 bins = border-clamped shifts of pool0 (9 bins, offsets +-1) and
  pool1 (8 bins, offsets +-3); bins 17..28 of 29 are zero.
  out = [29*384, 37, 37] with channel c = bin*384 + feature.

Sharding: pure data parallel, sample b -> core b (B == 8 == n_cores).

Device strategy (per core), built to keep the store-DMA stream (36 MB) the
only critical path:
  - channels on partitions (3 tiles of 128), spatial flattened in free dim
  - for each (pool k, dx) a column-pre-shifted, row-replicated-padded plane
    R[k][dxi][t]; every bin is then a CONTIGUOUS row-window of one plane, so
    bin stores are pure DMAs (grouped per (dy, ctile), 2-3 bins each)
  - R0 planes are filled by offset input DMAs (flat column shift == +-1
    element offset in DRAM), only edge-column fixes run on engines
  - pool1 separable 3x3 sum uses FLAT shifted adds + tiny edge-column fixes
    (strided big ops are ~3x slower than flat); the S*inv multiply runs once
    per dx with flat-shifted operands, writing each R1 plane directly
  - row padding comes from a padded S_pad + host-padded inv_pad, so pad rows
    cost 6 small copies per ctile instead of per-plane fills
  - no stride-0 (broadcast) APs, no GpSimd data ops (both measured slow);
    GpSimd only issues input DMAs (SWDGE) so Sync's HWDGE FIFO is stores-only
  - the 12 zero bins are never written: ExternalOutput buffers are
    pre-zeroed by the runner on both the native and PJRT paths.
"""

import numpy as np

B = 8
W = 37          # spatial side
WW = W * W      # 1369
D = 384
P = 128
ST = D // P     # 3 channel tiles of 128
NBINS = 29
PAD0, PAD1 = 1, 3
R0ROWS = W + 2 * PAD0             # 39
R1ROWS = W + 2 * PAD1             # 43
R0F = R0ROWS * W                  # 1443 flat elems per plane
R1F = R1ROWS * W                  # 1591

_CACHE = {}


def _inv_count_padded() -> np.ndarray:
    """1/(3x3 valid count), row-padded to R1ROWS (pad rows replicate edge
    rows), tiled to [P, R1F]."""
    cnt = np.zeros((W, W), np.float64)
    for dy in (-1, 0, 1):
        for dx in (-1, 0, 1):
            cnt[max(0, dy) : W + min(0, dy), max(0, dx) : W + min(0, dx)] += 1.0
    inv = (1.0 / cnt).astype(np.float32)
    inv_pad = np.concatenate([np.repeat(inv[:1], PAD1, 0), inv, np.repeat(inv[-1:], PAD1, 0)])
    flat = inv_pad.reshape(R1F)
    return np.broadcast_to(flat, (P, R1F)).copy()


def _build_nc():
    import concourse.bass as bass  # noqa: F401
    import concourse.tile as tile
    from concourse import bacc, mybir
    from contextlib import ExitStack

    f32 = mybir.dt.float32
    nc = bacc.Bacc("TRN2", target_bir_lowering=False, debug=False)

    xt = nc.declare_dram_parameter("xt", [ST, P, WW], f32, isOutput=False)
    invp = nc.declare_dram_parameter("invp", [P, R1F], f32, isOutput=False)
    out = nc.declare_dram_parameter("out", [NBINS, ST, P, WW], f32, isOutput=True)

    with tile.TileContext(nc) as tc, ExitStack() as ctx:
        perm = ctx.enter_context(tc.tile_pool(name="perm", bufs=1))
        tmp = ctx.enter_context(tc.tile_pool(name="tmp", bufs=2))

        inv = perm.tile([P, R1F], f32, name="inv")
        nc.gpsimd.dma_start(inv[:, :], invp.ap())

        # R0: [dxi, t, 39, 37] (pad 1, dx in {-1,0,+1})
        # R1: [dxi, t, 43, 37] (pad 3, dx in {-3,0,+3})
        R0 = perm.tile([P, 3, ST, R0ROWS, W], f32, name="R0")
        R1 = perm.tile([P, 3, ST, R1ROWS, W], f32, name="R1")

        def r0flat(dxi, t):
            return R0[:, dxi, t].rearrange("p a b -> p (a b)")

        def r1flat(dxi, t):
            return R1[:, dxi, t].rearrange("p a b -> p (a b)")

        for t in range(ST):
            xf = xt.ap()[t]                      # [P, WW] DRAM
            # ---- R0 planes via offset DMAs (rows 0..38 = X rows -1..37
            # clamped; pad rows loaded shifted, then edge-column fixes) ----
            for dxi, dx in enumerate((-1, 0, 1)):
                f = r0flat(dxi, t)
                a, b = max(0, dx), max(0, -dx)   # src left-trim, dest left-trim
                # top pad row (X row 0), center rows, bottom pad row (X row 36)
                nc.gpsimd.dma_start(f[:, b : W - a], xf[:, a : W - b])
                nc.gpsimd.dma_start(f[:, W + b : W + WW - a], xf[:, a : WW - b])
                nc.gpsimd.dma_start(
                    f[:, W + WW + b : W + WW + W - a], xf[:, WW - W + a : WW - b]
                )
            # edge-column fixes (all 39 rows at once)
            X3 = R0[:, 1, t]                     # [P, 39, 37] padded x
            nc.scalar.copy(R0[:, 0, t, :, 0], X3[:, :, 0])
            nc.scalar.copy(R0[:, 2, t, :, W - 1], X3[:, :, W - 1])

            Xc = R0[:, 1, t, PAD0 : PAD0 + W, :].rearrange("p a b -> p (a b)")
            Xc3 = R0[:, 1, t, PAD0 : PAD0 + W, :]

            # ---- column pass: T[i,j] = sum_dx X[i, j+dx] (zero outside) ----
            T = tmp.tile([P, WW], f32, name="T", tag="T")
            T3 = T.rearrange("p (a b) -> p a b", a=W, b=W)
            nc.vector.tensor_add(T[:, 0 : WW - 1], Xc[:, 0 : WW - 1], Xc[:, 1:WW])
            nc.vector.tensor_copy(T[:, WW - 1 : WW], Xc[:, WW - 1 : WW])
            nc.vector.tensor_add(T[:, 1:WW], T[:, 1:WW], Xc[:, 0 : WW - 1])
            nc.vector.tensor_add(T3[:, :, 0], Xc3[:, :, 0], Xc3[:, :, 1])
            nc.vector.tensor_add(T3[:, :, W - 1], Xc3[:, :, W - 2], Xc3[:, :, W - 1])

            # ---- row pass into padded S (flat row shift is exact) ----
            Sp = tmp.tile([P, R1F], f32, name="Sp", tag="Sp")
            c0 = PAD1 * W                        # 111: center start in flat pad layout
            nW = WW - W
            nc.vector.tensor_add(Sp[:, c0 : c0 + nW], T[:, 0:nW], T[:, W:WW])
            nc.vector.tensor_copy(Sp[:, c0 + nW : c0 + WW], T[:, nW:WW])
            nc.vector.tensor_add(Sp[:, c0 + W : c0 + WW], Sp[:, c0 + W : c0 + WW], T[:, 0:nW])
            # pad rows: replicate first/last center row (contiguous copies)
            for i in range(PAD1):
                nc.vector.tensor_copy(Sp[:, i * W : (i + 1) * W], Sp[:, c0 : c0 + W])
                nc.vector.tensor_copy(
                    Sp[:, (PAD1 + W + i) * W : (PAD1 + W + i + 1) * W],
                    Sp[:, (PAD1 + W - 1) * W : (PAD1 + W) * W],
                )

            # ---- R1 planes: one flat-shifted multiply per dx ----
            for dxi, dx in enumerate((-3, 0, 3)):
                f = r1flat(dxi, t)
                a, b = max(0, dx), max(0, -dx)
                nc.vector.tensor_mul(
                    f[:, b : R1F - a],
                    Sp[:, a : R1F - b],
                    inv[:, a : R1F - b],
                )
            # edge-column fixes from the dx=0 plane (all 43 rows)
            P13 = R1[:, 1, t]
            for c in range(PAD1):
                nc.scalar.copy(R1[:, 0, t, :, c], P13[:, :, 0])
                nc.scalar.copy(R1[:, 2, t, :, W - 1 - c], P13[:, :, W - 1])

            # ---- stores: one DMA per (dy-group, ctile) ----
            for r_i, dy in enumerate((-1, 0, 1)):
                src = R0[:, :, t, PAD0 + dy : PAD0 + dy + W, :].rearrange(
                    "p x a b -> p x (a b)"
                )
                dst = out.ap()[3 * r_i : 3 * r_i + 3, t].transpose([1, 0, 2])
                nc.sync.dma_start(dst, src)
            for dy, dxis, p0 in ((-3, (0, 1, 2), 9), (0, (0,), 12), (0, (2,), 13), (3, (0, 1, 2), 14)):
                lo = PAD1 + dy
                if len(dxis) == 3:
                    src = R1[:, :, t, lo : lo + W, :].rearrange("p x a b -> p x (a b)")
                    dst = out.ap()[p0 : p0 + 3, t].transpose([1, 0, 2])
                else:
                    src = R1[:, dxis[0], t, lo : lo + W, :].rearrange("p a b -> p (a b)")
                    dst = out.ap()[p0, t]
                nc.sync.dma_start(dst, src)

    nc.compile()
    return nc


def get_nc():
    if "nc" not in _CACHE:
        _CACHE["nc"] = _build_nc()
    return _CACHE["nc"]


def make_in_maps(x: np.ndarray):
    x = np.ascontiguousarray(x, dtype=np.float32)
    assert x.shape == (B, W, W, D), x.shape
    inv = _inv_count_padded()
    maps = []
    for b in range(B):
        xtr = x[b].transpose(2, 0, 1).reshape(ST, P, WW)
        maps.append({"xt": np.ascontiguousarray(xtr), "invp": inv})
    return maps


def run(x: np.ndarray, **kw):
    from concourse.bass_utils import run_bass_kernel_spmd

    nc = get_nc()
    res = run_bass_kernel_spmd(nc, make_in_maps(x), core_ids=list(range(B)), **kw)
    outs = np.stack([res.results[b]["out"].reshape(NBINS * D, W, W) for b in range(B)])
    return outs, res


def kernel(x: np.ndarray) -> np.ndarray:
    outs, _ = run(x)
    return outs
